# revision 15
# baseline (speedup 1.0000x reference)
"""Trainium2 Bass kernel for nn_NeuralTensorDiagLayer.

Computes out = tanh(concat([e1, e2], -1) @ V + diag + b) where
diag[k] = (sum_b(e1*e2) @ W[k]) / (B*D), broadcast over batch.

Sharding (8 NeuronCores):
  - Data-parallel over batch: core c handles rows [c*512, (c+1)*512).
  - Each core streams the full V (feature-major, so no device transpose);
    e1/e2 shards are fed pre-transposed to [feat, batch] by the host.
  - diag: each core computes a partial sum_b(e1*e2) over its batch shard
    (fused DVE mul+reduce along the free axis), AllReduce(8 KiB), then a
    tiny matvec against its 256-row slice of W (fed as W^T), AllGather the
    2048-dim diag (2 KiB), and applies it as the per-partition activation
    bias of the tanh epilogue.
  - Main matmul runs in float32r (TensorE reduced-precision fp32 mode,
    4x the fp32 throughput at ~12-bit mantissa accuracy); accumulation is
    fp32 in PSUM. The tiny diag matvec runs in plain fp32.

Output is produced transposed ([k_out, batch] per core); the host
transposes/concats back to (B, K).
"""

import os
import sys

for _p in ("/opt/trn_rl_repo", "/root/.axon_site/_ro/trn_rl_repo"):
    if os.path.isdir(_p) and _p not in sys.path:
        sys.path.append(_p)

import numpy as np

N_CORES = 8
B, D, K_OUT = 4096, 2048, 2048
FEAT = 2 * D                   # 4096 contraction dim
BPC = B // N_CORES             # 512 batch rows per core
KPC = K_OUT // N_CORES         # 256 diag rows per core
FT = FEAT // 128               # 32 feature tiles
DT = D // 128                  # 16 prod/feature tiles (e1-space)
KT = K_OUT // 128              # 16 k_out tiles
KG, KPG = 4, 4                 # k_out processed in 4 groups of 4 tiles
DIAG_SCALE = 1.0 / (B * D)

_CACHE = {}


def _build_nc():
    import concourse.bacc as bacc
    import concourse.tile as tile
    import concourse.mybir as mybir

    no_diagmm = bool(int(os.environ.get("KERNEL_NO_DIAGMM", "0")))
    no_cc = bool(int(os.environ.get("KERNEL_NO_CC", "0")))
    no_main = bool(int(os.environ.get("KERNEL_NO_MAIN", "0")))
    repeat = int(os.environ.get("KERNEL_REPEAT", "1"))

    dt = mybir.dt
    nc = bacc.Bacc("TRN2", target_bir_lowering=False, debug=False,
                   num_devices=N_CORES)

    e1t = nc.dram_tensor("e1t", [D, BPC], dt.float32r, kind="ExternalInput").ap()
    e2t = nc.dram_tensor("e2t", [D, BPC], dt.float32r, kind="ExternalInput").ap()
    v = nc.dram_tensor("v", [FEAT, K_OUT], dt.float32r, kind="ExternalInput").ap()
    wt = nc.dram_tensor("wt", [D, KPC], dt.float32, kind="ExternalInput").ap()
    bvec = nc.dram_tensor("bvec", [1, KPC], dt.float32, kind="ExternalInput").ap()
    out = nc.dram_tensor("out", [K_OUT, BPC], dt.float32, kind="ExternalOutput").ap()

    core_ids = list(range(N_CORES))

    with tile.TileContext(nc) as tc:
        with tc.tile_pool(name="xpool", bufs=1) as xpool, \
             tc.tile_pool(name="vpool", bufs=4) as vpool, \
             tc.tile_pool(name="wpool", bufs=4) as wpool, \
             tc.tile_pool(name="spool", bufs=1) as spool, \
             tc.tile_pool(name="scratch", bufs=3) as scratch, \
             tc.tile_pool(name="stage", bufs=1) as stage_pool, \
             tc.tile_pool(name="opool", bufs=4) as opool, \
             tc.tile_pool(name="psum", bufs=7, space="PSUM") as pp, \
             tc.tile_pool(name="psd", bufs=1, space="PSUM") as ppd, \
             tc.tile_pool(name="dram", bufs=1, space="DRAM") as dram:

            # ---- resident X^T = [e1^T ; e2^T] : 32 tiles of [128, BPC] ----
            x_all = xpool.tile([128, FT * BPC], dt.float32r)
            for j in range(DT):
                nc.sync.dma_start(x_all[:, j * BPC:(j + 1) * BPC],
                                  e1t[j * 128:(j + 1) * 128, :])
            for j in range(DT):
                jj = DT + j
                nc.sync.dma_start(x_all[:, jj * BPC:(jj + 1) * BPC],
                                  e2t[j * 128:(j + 1) * 128, :])

            # ---- partial s = sum_batch(e1*e2) on DVE (fused mul+reduce) ----
            s_sb = spool.tile([128, DT], dt.float32)
            for j in range(DT):
                prod = scratch.tile([128, BPC], dt.float32, tag="prod", name=f"prod{j}")
                nc.vector.tensor_mul(
                    prod[:],
                    x_all[:, j * BPC:(j + 1) * BPC].bitcast(dt.float32),
                    x_all[:, (DT + j) * BPC:(DT + j + 1) * BPC].bitcast(dt.float32))
                nc.vector.tensor_reduce(s_sb[:, j:j + 1], prod[:],
                                        mybir.AxisListType.X,
                                        mybir.AluOpType.add)

            # ---- AllReduce s over the batch shards (8 KiB) ----
            s_in = dram.tile([128, DT], dt.float32)
            s_out = dram.tile([128, DT], dt.float32,
                              addr_space="Local" if no_cc else "Shared")
            nc.sync.dma_start(s_in[:], s_sb[:])
            if no_cc:
                nc.sync.dma_start(s_out[:], s_in[:])
            else:
                nc.gpsimd.collective_compute(
                    "AllReduce", mybir.AluOpType.add,
                    replica_groups=[core_ids],
                    ins=[s_in.opt()], outs=[s_out.opt()])
            s_r = spool.tile([128, DT], dt.float32, name="s_r")
            nc.sync.dma_start(s_r[:], s_out[:])

            # ---- diag slice: [1, KPC] = s @ wt (fp32 matmuls, M=1) ----
            b_sb = spool.tile([1, KPC], dt.float32, name="b_sb")
            nc.sync.dma_start(b_sb[:], bvec[:])
            diag_sb = spool.tile([1, KPC], dt.float32, name="diag_sb")
            if no_diagmm:
                nc.vector.tensor_scalar_mul(diag_sb[:], s_r[0:1, 0:1].broadcast_to((1, KPC)), 0.0)
                nc.vector.tensor_add(diag_sb[:], diag_sb[:], b_sb[:])
            else:
                ps_d = ppd.tile([1, KPC], dt.float32)
                for j in range(DT):
                    wt_t = wpool.tile([128, KPC], dt.float32, tag="wt", name=f"wt{j}")
                    nc.sync.dma_start(wt_t[:], wt[j * 128:(j + 1) * 128, :])
                    nc.tensor.matmul(ps_d[:], s_r[:, j:j + 1], wt_t[:],
                                     start=(j == 0), stop=(j == DT - 1))
                nc.vector.tensor_scalar_mul(diag_sb[:], ps_d[:], DIAG_SCALE)
                nc.vector.tensor_add(diag_sb[:], diag_sb[:], b_sb[:])

            # ---- AllGather diag slices -> full [K_OUT] (2 KiB) ----
            d_in = dram.tile([1, KPC], dt.float32, name="d_in")
            d_out = dram.tile([KT, 128], dt.float32,
                              addr_space="Local" if no_cc else "Shared",
                              name="d_out")
            nc.sync.dma_start(d_in[:], diag_sb[:])
            if no_cc:
                for i in range(N_CORES):
                    nc.sync.dma_start(d_out[2 * i:2 * i + 2, :],
                                      d_in[:].rearrange("a (x p) -> (a x) p", p=128))
            else:
                nc.gpsimd.collective_compute(
                    "AllGather", mybir.AluOpType.bypass,
                    replica_groups=[core_ids],
                    ins=[d_in.opt()], outs=[d_out.opt()])
            # load as [128, KT]: partition p, col k  <-  diag[k*128 + p]
            diag_cols = spool.tile([128, KT], dt.float32, name="diag_cols")
            nc.sync.dma_start(diag_cols[:], d_out[:].rearrange("k p -> p k"))

            # ---- main matmul: out^T[k,b] = V^T @ X^T, f32r on TensorE ----
            stage = stage_pool.tile([128, KT * BPC], dt.float32)
            for _rep in range(repeat):
              for kg in range(0 if not no_main else KG, KG):
                pss = [pp.tile([128, BPC], dt.float32, tag="ps", name=f"ps{kg}_{q}")
                       for q in range(KPG)]
                for j in range(FT):
                    vt = vpool.tile([128, KPG * 128], dt.float32r, tag="vt")
                    nc.sync.dma_start(
                        vt[:], v[j * 128:(j + 1) * 128,
                                 kg * KPG * 128:(kg + 1) * KPG * 128])
                    for q in range(KPG):
                        nc.tensor.matmul(pss[q][:],
                                         vt[:, q * 128:(q + 1) * 128],
                                         x_all[:, j * BPC:(j + 1) * BPC],
                                         start=(j == 0), stop=(j == FT - 1))
                for q in range(KPG):
                    k = kg * KPG + q
                    # copy out of PSUM on DVE so PE never waits on the
                    # diag collective chain; tanh+bias runs on ACT after.
                    nc.vector.tensor_copy(stage[:, k * BPC:(k + 1) * BPC],
                                          pss[q][:])
                    ot = opool.tile([128, BPC], dt.float32, tag="ot",
                                    name=f"ot{k}")
                    nc.scalar.activation(ot[:], stage[:, k * BPC:(k + 1) * BPC],
                                         mybir.ActivationFunctionType.Tanh,
                                         bias=diag_cols[:, k:k + 1])
                    nc.sync.dma_start(out[k * 128:(k + 1) * 128, :], ot[:])

    nc.compile()
    return nc


def _get_nc():
    if "nc" not in _CACHE:
        _CACHE["nc"] = _build_nc()
    return _CACHE["nc"]


def kernel(e1, e2, W, V, b):
    from concourse.bass_utils import run_bass_kernel_spmd

    e1 = np.asarray(e1, dtype=np.float32)
    e2 = np.asarray(e2, dtype=np.float32)
    W = np.asarray(W, dtype=np.float32)
    V = np.asarray(V, dtype=np.float32)
    b = np.asarray(b, dtype=np.float32)

    nc = _get_nc()
    in_maps = []
    for c in range(N_CORES):
        rows = slice(c * BPC, (c + 1) * BPC)
        krows = slice(c * KPC, (c + 1) * KPC)
        in_maps.append({
            "e1t": np.ascontiguousarray(e1[rows].T),
            "e2t": np.ascontiguousarray(e2[rows].T),
            "v": V,
            "wt": np.ascontiguousarray(W[krows].T),
            "bvec": b[krows].reshape(1, KPC),
        })
    res = run_bass_kernel_spmd(nc, in_maps, list(range(N_CORES)))
    out = np.empty((B, K_OUT), dtype=np.float32)
    for c in range(N_CORES):
        out[c * BPC:(c + 1) * BPC, :] = res.results[c]["out"].T
    return out
